# revision 1
# baseline (speedup 1.0000x reference)
"""GATConv (8 heads, 16 ch/head, self-loops, segment softmax) on 8 TRN2 NeuronCores.

Strategy (graph/data parallel, per sharding hint):
  - dst-node ranges are sharded across the 8 cores (12500 dst nodes each).
  - Host packs each core's incoming edges (sorted by dst) into 128-edge
    "tiles"; each tile covers <= 16 consecutive dst nodes and no dst's edge
    segment crosses a tile boundary.  Per-tile metadata (src ids, dst ids,
    local dst slot) is shipped as tensors so one uniform program runs on all
    cores (SPMD).
  - Device phase A (replicated): h = x @ W.T plus the folded per-node
    attention logits a_src/a_dst via one matmul against [W.T | Wa_src |
    Wa_dst]; writes hX=[h|a_src] and aD=a_dst to DRAM.
  - Device phase B: per 128-edge tile, indirect-DMA gather of hX rows by src
    id and aD rows by dst id; w = exp(leakyrelu(a_src+a_dst)) (softmax max-
    subtraction is skipped: |logits| < ~6 so exp is safe and the result is
    mathematically identical); one-hot S[e, j] = (jloc==j) built with
    is_equal vs iota; one matmul S.T @ [w*h | w] accumulates numerator and
    denominator per dst slot into num_den in DRAM.
  - Device phase C: divide, add bias, indirect-DMA scatter rows to the
    core's output shard (padded rows dropped via bounds_check).
"""

import numpy as np

import concourse.bass as bass
import concourse.bacc as bacc
import concourse.mybir as mybir
import concourse.tile as tile

P = 128
HEADS = 8
F_OUT = 16
HF = 128
C_IN = 128
NEG_SLOPE = 0.2

N_NODES = 100000
N_EDGES = 1600000
N_CORES = 8

EPT = 128   # edge slots per tile
DPT = 32    # max dst nodes per tile
TB = 12     # tiles per phase-B batch (multiple of 3: 3 tiles share a PSUM group)
CPG = 2     # phase-C 128-row chunks per group
PCG = 9     # phase-C groups per For_i iteration
XB = 8      # phase-A 128-node blocks per x_T load
PAD_J = 999.0
USE_FORI = False  # hardware loops only needed if walrus sem accounting overflows

f32 = mybir.dt.float32
bf16 = mybir.dt.bfloat16
i32 = mybir.dt.int32


# ---------------------------------------------------------------- host side

def _pack_core(src, dst, lo, hi):
    """Pack one core's edges (dst in [lo, hi)) into 128-edge tiles."""
    npc = hi - lo
    order = np.argsort(dst, kind="stable")
    s = src[order].astype(np.int32)
    d = dst[order].astype(np.int32)
    dloc = (d - lo).astype(np.int64)
    counts = np.bincount(dloc, minlength=npc)
    estarts = np.concatenate([[0], np.cumsum(counts)])

    tiles = []  # (d0, k, e0, ecnt)
    t_d0, t_k, t_e = 0, 0, 0
    for dd in range(npc):
        g = int(counts[dd])
        assert 0 < g <= EPT, f"dst degree {g} exceeds tile capacity"
        if t_k > 0 and (t_e + g > EPT or t_k >= DPT):
            tiles.append((t_d0, t_k, int(estarts[t_d0]), t_e))
            t_k, t_e = 0, 0
        if t_k == 0:
            t_d0 = dd
        t_k += 1
        t_e += g
    tiles.append((t_d0, t_k, int(estarts[t_d0]), t_e))

    ntr = len(tiles)
    src_idx = np.zeros((P, ntr), np.int32)
    dst_idx = np.zeros((P, ntr), np.int32)
    jloc = np.full((P, ntr), PAD_J, np.float32)
    d0s = np.zeros(ntr, np.int64)
    ks = np.zeros(ntr, np.int64)
    for t, (d0, k, e0, ec) in enumerate(tiles):
        src_idx[:ec, t] = s[e0 : e0 + ec]
        dst_idx[:ec, t] = d[e0 : e0 + ec]
        jloc[:ec, t] = (dloc[e0 : e0 + ec] - d0).astype(np.float32)
        d0s[t] = d0
        ks[t] = k
    return src_idx, dst_idx, jloc, d0s, ks


def _preprocess(x, W, att, bias, edge_index, n_nodes, n_cores):
    npc = n_nodes // n_cores
    src_all = np.asarray(edge_index[0])
    dst_all = np.asarray(edge_index[1])
    loops = np.arange(n_nodes, dtype=src_all.dtype)
    src_all = np.concatenate([src_all, loops])
    dst_all = np.concatenate([dst_all, loops])

    packed = []
    for r in range(n_cores):
        lo, hi = r * npc, (r + 1) * npc
        m = (dst_all >= lo) & (dst_all < hi)
        packed.append(_pack_core(src_all[m], dst_all[m], lo, hi))

    NT = max(p[0].shape[1] for p in packed)
    NTQ = 72  # lcm(TB=12, phase-C iteration granule CPG*PCG*P/DPT=72)
    NT = ((NT + NTQ - 1) // NTQ) * NTQ
    NC = NT * DPT // P

    # shared tensors
    npad = ((n_nodes + P - 1) // P) * P
    x_T = np.zeros((C_IN, npad), np.float32)
    x_T[:, :n_nodes] = np.asarray(x, np.float32).T
    W = np.asarray(W, np.float32)
    att = np.asarray(att, np.float32)
    att_src = att[:HF].reshape(HEADS, F_OUT)
    att_dst = att[HF:].reshape(HEADS, F_OUT)
    Wr = W.reshape(HEADS, F_OUT, C_IN)
    Wa_src = np.einsum("hfc,hf->ch", Wr, att_src).astype(np.float32)
    Wa_dst = np.einsum("hfc,hf->ch", Wr, att_dst).astype(np.float32)
    Wcat = np.ascontiguousarray(
        np.concatenate([W.T, Wa_src, Wa_dst], axis=1), np.float32
    )
    iota16 = np.tile(np.arange(DPT, dtype=np.float32), (P, 1))
    brep = np.tile(np.asarray(bias, np.float32)[None, :], (P, 1))

    in_maps = []
    for r in range(n_cores):
        src_idx, dst_idx, jloc, d0s, ks = packed[r]
        ntr = src_idx.shape[1]
        if ntr < NT:
            pad = NT - ntr
            src_idx = np.pad(src_idx, ((0, 0), (0, pad)))
            dst_idx = np.pad(dst_idx, ((0, 0), (0, pad)))
            jloc = np.pad(jloc, ((0, 0), (0, pad)), constant_values=PAD_J)
        # num_den row r -> output row (core-local dst), or npc to drop
        rows = np.arange(NT * DPT)
        tl, j = rows // DPT, rows % DPT
        ks_pad = np.zeros(NT, np.int64)
        ks_pad[:ntr] = ks
        d0s_pad = np.zeros(NT, np.int64)
        d0s_pad[:ntr] = d0s
        val = np.where(j < ks_pad[tl], d0s_pad[tl] + j, npc)
        out_row = np.ascontiguousarray(
            val.reshape(NT * DPT // P, P).T.astype(np.int32)
        )
        in_maps.append(
            {
                "x_T": x_T,
                "Wcat": Wcat,
                "iota16": iota16,
                "brep": brep,
                "src_idx": np.ascontiguousarray(src_idx),
                "dst_idx": np.ascontiguousarray(dst_idx),
                "jloc": np.ascontiguousarray(jloc),
                "out_row": out_row,
            }
        )
    return in_maps, NT, NC, npc, npad


# -------------------------------------------------------------- device side

def build_program(n_nodes, npc, npad, NT, NC, dbg=False):
    nblk = npad // P
    nc = bacc.Bacc("TRN2", target_bir_lowering=False, debug=False)
    dbg_kind = {"kind": "ExternalOutput"} if dbg else {}

    x_T = nc.dram_tensor("x_T", [C_IN, npad], f32, kind="ExternalInput")
    Wcat = nc.dram_tensor("Wcat", [C_IN, HF + 2 * HEADS], f32, kind="ExternalInput")
    iota16 = nc.dram_tensor("iota16", [P, DPT], f32, kind="ExternalInput")
    brep = nc.dram_tensor("brep", [P, HF], f32, kind="ExternalInput")
    src_idx = nc.dram_tensor("src_idx", [P, NT], i32, kind="ExternalInput")
    dst_idx = nc.dram_tensor("dst_idx", [P, NT], i32, kind="ExternalInput")
    jloc = nc.dram_tensor("jloc", [P, NT], f32, kind="ExternalInput")
    out_row = nc.dram_tensor("out_row", [P, NC], i32, kind="ExternalInput")

    hX = nc.dram_tensor("hX", [npad, HF + HEADS], f32, **dbg_kind)
    aD = nc.dram_tensor("aD", [npad, HEADS], f32, **dbg_kind)
    num_den = nc.dram_tensor("num_den", [NT * DPT, HF + HEADS], f32, **dbg_kind)
    out = nc.dram_tensor("out", [npc, HF], f32, kind="ExternalOutput")

    W144 = HF + 2 * HEADS   # 144
    W136 = HF + HEADS       # 136

    with tile.TileContext(nc) as tc:
        with (
            tc.tile_pool(name="const", bufs=1) as cpool,
            tc.tile_pool(name="work", bufs=3) as wpool,
            tc.tile_pool(name="psA", bufs=4, space="PSUM") as psA,
            tc.tile_pool(name="psB", bufs=4, space="PSUM") as psB,
        ):
            wc = cpool.tile([C_IN, W144], f32, tag="wc")
            nc.sync.dma_start(out=wc[:], in_=Wcat[:])
            io16 = cpool.tile([P, DPT], f32, tag="io16")
            nc.sync.dma_start(out=io16[:], in_=iota16[:])
            bi = cpool.tile([P, HF], f32, tag="bi")
            nc.sync.dma_start(out=bi[:], in_=brep[:])

            # ---------------- phase A: hX = [x@Wcat] ----------------
            # 3 node-blocks share one PSUM bank (3*144 = 432 f32 <= 512),
            # copied to SBUF in one op (DMA cannot read PSUM).
            AB = 3
            for c0 in range(0, nblk, XB):
                nb = min(XB, nblk - c0)
                xt = wpool.tile([C_IN, nb * P], f32, tag="xt")
                nc.sync.dma_start(
                    out=xt[:], in_=x_T[:, c0 * P : (c0 + nb) * P]
                )
                for g0 in range(0, nb, AB):
                    ng = min(AB, nb - g0)
                    ps = psA.tile([P, AB * W144], f32, tag="psA")
                    for b in range(ng):
                        nc.tensor.matmul(
                            out=ps[:, b * W144 : (b + 1) * W144],
                            lhsT=xt[:, (g0 + b) * P : (g0 + b + 1) * P],
                            rhs=wc[:],
                            start=True,
                            stop=True,
                        )
                    ha = wpool.tile([P, AB * W144], f32, tag="ha")
                    nc.vector.tensor_copy(
                        out=ha[:, : ng * W144], in_=ps[:, : ng * W144]
                    )
                    har = ha[:].rearrange("p (q c) -> p q c", q=AB)
                    r0 = (c0 + g0) * P
                    nc.sync.dma_start(
                        out=hX[r0 : r0 + ng * P, :].rearrange(
                            "(q p) c -> p q c", p=P
                        ),
                        in_=har[:, :ng, :W136],
                    )
                    nc.sync.dma_start(
                        out=aD[r0 : r0 + ng * P, :].rearrange(
                            "(q p) c -> p q c", p=P
                        ),
                        in_=har[:, :ng, W136:W144],
                    )

            def _s(x, n):
                return bass.ds(x, n) if not isinstance(x, int) else slice(x, x + n)

            # ---------------- phase B: edge tiles -------------------
            def phase_b(bv):
                sid = wpool.tile([P, TB], i32, tag="sid")
                nc.sync.dma_start(out=sid[:], in_=src_idx[:, _s(bv, TB)])
                did = wpool.tile([P, TB], i32, tag="did")
                nc.sync.dma_start(out=did[:], in_=dst_idx[:, _s(bv, TB)])
                jl = wpool.tile([P, TB], f32, tag="jl")
                nc.sync.dma_start(out=jl[:], in_=jloc[:, _s(bv, TB)])

                # walrus supports exactly one offset per partition per
                # indirect DMA, so gather tile-by-tile ([128,1] offsets).
                hg = wpool.tile([P, TB * W136], f32, tag="hg")
                ag = wpool.tile([P, TB * HEADS], f32, tag="ag")
                for t in range(TB):
                    nc.gpsimd.indirect_dma_start(
                        out=hg[:, t * W136 : (t + 1) * W136],
                        out_offset=None,
                        in_=hX[:, :],
                        in_offset=bass.IndirectOffsetOnAxis(
                            ap=sid[:, t : t + 1], axis=0
                        ),
                    )
                    nc.gpsimd.indirect_dma_start(
                        out=ag[:, t * HEADS : (t + 1) * HEADS],
                        out_offset=None,
                        in_=aD[:, :],
                        in_offset=bass.IndirectOffsetOnAxis(
                            ap=did[:, t : t + 1], axis=0
                        ),
                    )

                hgr = hg[:].rearrange("p (t c) -> p t c", t=TB)
                # e = a_src[src] + a_dst[dst]; lrelu = max(e, 0.2e); w = exp
                e4 = wpool.tile([P, TB * HEADS], f32, tag="e4")
                nc.vector.tensor_tensor(
                    out=e4[:].rearrange("p (t h) -> p t h", t=TB),
                    in0=hgr[:, :, HF:W136],
                    in1=ag[:].rearrange("p (t h) -> p t h", t=TB),
                    op=mybir.AluOpType.add,
                )
                e5 = wpool.tile([P, TB * HEADS], f32, tag="e5")
                nc.vector.tensor_scalar_mul(out=e5[:], in0=e4[:], scalar1=NEG_SLOPE)
                nc.vector.tensor_tensor(
                    out=e5[:], in0=e4[:], in1=e5[:], op=mybir.AluOpType.max
                )
                wf = wpool.tile([P, TB * HEADS], f32, tag="wf")
                nc.scalar.activation(
                    out=wf[:], in_=e5[:], func=mybir.ActivationFunctionType.Exp
                )

                # msg buffer: [w*h | w] per tile, bf16 for the PE
                mbuf = wpool.tile([P, TB * W136], bf16, tag="mbuf")
                mbr = mbuf[:].rearrange("p (t c) -> p t c", t=TB)
                wfr4 = (
                    wf[:]
                    .rearrange("p (t h o) -> p t h o", h=HEADS, o=1)
                    .to_broadcast([P, TB, HEADS, F_OUT])
                )
                nc.vector.tensor_tensor(
                    out=mbr[:, :, :HF].rearrange("p t (h f) -> p t h f", h=HEADS),
                    in0=hgr[:, :, :HF].rearrange("p t (h f) -> p t h f", h=HEADS),
                    in1=wfr4,
                    op=mybir.AluOpType.mult,
                )
                nc.vector.tensor_copy(
                    out=mbr[:, :, HF:W136],
                    in_=wf[:].rearrange("p (t h) -> p t h", t=TB),
                )

                # one-hot S[e, j] = (jloc == j)
                S = wpool.tile([P, TB * DPT], bf16, tag="S")
                nc.vector.tensor_tensor(
                    out=S[:].rearrange("p (t j) -> p t j", t=TB),
                    in0=jl[:]
                    .rearrange("p (t o) -> p t o", o=1)
                    .to_broadcast([P, TB, DPT]),
                    in1=io16[:]
                    .rearrange("p (o j) -> p o j", o=1)
                    .to_broadcast([P, TB, DPT]),
                    op=mybir.AluOpType.is_equal,
                )

                # 3 tiles (32 dst slots each) share one [96, 136] PSUM
                # tile at partition bases 0/32/64 (the only legal bases).
                for g in range(TB // 3):
                    nd = psB.tile([3 * DPT, W136], f32, tag="psB")
                    for i in range(3):
                        ti = g * 3 + i
                        nc.tensor.matmul(
                            out=nd[i * DPT : (i + 1) * DPT, :],
                            lhsT=S[:, ti * DPT : (ti + 1) * DPT],
                            rhs=mbuf[:, ti * W136 : (ti + 1) * W136],
                            start=True,
                            stop=True,
                        )
                    nds = wpool.tile([3 * DPT, W136], f32, tag="nds")
                    nc.vector.tensor_copy(out=nds[:], in_=nd[:])
                    nc.sync.dma_start(
                        out=num_den[_s(bv * DPT + g * 3 * DPT, 3 * DPT), :],
                        in_=nds[:],
                    )

            if USE_FORI:
                with tc.For_i(0, NT, TB, staggered_reset=True) as bv:
                    phase_b(bv)
            else:
                for bv in range(0, NT, TB):
                    phase_b(bv)

            # ---------------- phase C: divide + scatter -------------
            def phase_c(cv, gg):
                ndt = wpool.tile([P, CPG * W136], f32, tag="ndt")
                nc.sync.dma_start(
                    out=ndt[:].rearrange("p (q c) -> p q c", q=CPG),
                    in_=num_den[
                        _s((cv + gg * CPG) * P, CPG * P), :
                    ].rearrange("(q p) c -> p q c", p=P),
                )
                ndr = ndt[:].rearrange("p (q c) -> p q c", q=CPG)
                rc = wpool.tile([P, CPG * HEADS], f32, tag="rc")
                rcr = rc[:].rearrange("p (q h) -> p q h", q=CPG)
                nc.vector.tensor_scalar_add(
                    out=rcr, in0=ndr[:, :, HF:W136], scalar1=1e-30
                )
                nc.vector.reciprocal(out=rc[:], in_=rc[:])
                ot = wpool.tile([P, CPG * HF], f32, tag="ot")
                otr = ot[:].rearrange("p (q c) -> p q c", q=CPG)
                nc.vector.tensor_tensor(
                    out=otr.rearrange("p q (h f) -> p q h f", h=HEADS),
                    in0=ndr[:, :, :HF].rearrange("p q (h f) -> p q h f", h=HEADS),
                    in1=rc[:]
                    .rearrange("p (q h o) -> p q h o", h=HEADS, o=1)
                    .to_broadcast([P, CPG, HEADS, F_OUT]),
                    op=mybir.AluOpType.mult,
                )
                nc.vector.tensor_tensor(
                    out=otr,
                    in0=otr,
                    in1=bi[:]
                    .rearrange("p (o c) -> p o c", o=1)
                    .to_broadcast([P, CPG, HF]),
                    op=mybir.AluOpType.add,
                )
                orw = wpool.tile([P, CPG], i32, tag="orw")
                nc.sync.dma_start(
                    out=orw[:], in_=out_row[:, _s(cv + gg * CPG, CPG)]
                )
                for q in range(CPG):
                    nc.gpsimd.indirect_dma_start(
                        out=out[:, :],
                        out_offset=bass.IndirectOffsetOnAxis(
                            ap=orw[:, q : q + 1], axis=0
                        ),
                        in_=ot[:, q * HF : (q + 1) * HF],
                        in_offset=None,
                        bounds_check=npc - 1,
                        oob_is_err=False,
                    )

            if USE_FORI:
                with tc.For_i(0, NC, CPG * PCG, staggered_reset=True) as cv:
                    for gg in range(PCG):
                        phase_c(cv, gg)
            else:
                for cv in range(0, NC, CPG * PCG):
                    for gg in range(PCG):
                        phase_c(cv, gg)
    nc.compile()
    return nc


# ------------------------------------------------------------- entry point

_CACHE = {}


def kernel(x, W, att, bias, edge_index):
    from concourse.bass_utils import run_bass_kernel_spmd

    n_nodes = x.shape[0]
    in_maps, NT, NC, npc, npad = _preprocess(
        x, W, att, bias, edge_index, n_nodes, N_CORES
    )
    key = (n_nodes, NT)
    if key not in _CACHE:
        _CACHE[key] = build_program(n_nodes, npc, npad, NT, NC)
    nc = _CACHE[key]
    res = run_bass_kernel_spmd(nc, in_maps, list(range(N_CORES))).results
    out = np.concatenate([res[r]["out"] for r in range(N_CORES)], axis=0)
    return out.astype(np.float32)

